# revision 2
# baseline (speedup 1.0000x reference)
"""CausalAttention (B=4, T=2048, C=1024, H=16, D=64) on 8 TRN2 NeuronCores.

Sharding: core c -> (batch b = c//2, head-group hg = c%2 covering heads
hg*8..hg*8+7).  Each core computes QKV for its batch restricted to its 8
heads, causal attention, and the output projection over the AllGathered
at for half the rows of each q-chunk.

Device algorithm (per core, all matmuls bf16):
  QKV units: qkT[j,t] = Wqk^T x^T   (Q,K kept transposed: [channels, T])
             V[t,j]   = x Wv        (stored with a ones-column per head)
  attention per q-chunk of 512, per head-pair:
           sT[k,q] = K_h^T q-block (scores transposed; causal blocks only)
           diag slabs: sT += -1e9 * tri  (mask folded into PSUM accum)
           expT = exp(0.125*sT)          (ACT, PSUM->SBUF bf16)
           out'[d,q] (+ sumexp in row 64) = V'_h^T @ expT (accumulate over k)
           at[c,q] = out'[0:64]/sumexp  (recip + partition_broadcast + mul)
  exchange: AllGather(at) over the core pair via DRAM staging
  proj:     at^T Wproj + bias (per 128-row tile, my 256 rows per chunk)

Schedule: QKV units and proj pieces are interleaved into the attention
chunks at head-pair granularity so the PE never starves while ACT (exp)
runs; input DMA issues are spread across engine queues.
"""
import ml_dtypes
import numpy as np

import concourse.bass as bass
import concourse.tile as tile
from concourse import bacc, mybir
from concourse.bass_utils import run_bass_kernel_spmd

F32 = mybir.dt.float32
AF = mybir.ActivationFunctionType

B, T, C = 4, 2048, 1024
H, D = 16, 64
HL = 8           # heads per core
CL = HL * D      # local channels (512)
CDT = mybir.dt.bfloat16  # matmul compute dtype
QC = 512         # q-chunk width
NQC = T // QC    # 4
KT = 128         # k-tile
N5 = 512         # matmul free-dim / PSUM bank width (fp32)


def _build():
    nc = bacc.Bacc("TRN2", target_bir_lowering=False, debug=False, num_devices=8)

    xT = nc.dram_tensor("xT", [8, 128, T], CDT, kind="ExternalInput").ap()
    wqk = nc.dram_tensor("wqk", [8, 128, 1024], CDT, kind="ExternalInput").ap()
    wv = nc.dram_tensor("wv", [8, 128, CL], CDT, kind="ExternalInput").ap()
    wproj = nc.dram_tensor("wproj", [8, 128, C], CDT, kind="ExternalInput").ap()
    bias2 = nc.dram_tensor("bias2", [1, C], CDT, kind="ExternalInput").ap()
    ones_r = nc.dram_tensor("ones_r", [1, 128], CDT, kind="ExternalInput").ap()
    ident = nc.dram_tensor("ident", [128, 128], CDT, kind="ExternalInput").ap()
    negtri = nc.dram_tensor("negtri", [128, 128], CDT, kind="ExternalInput").ap()
    vones = nc.dram_tensor("vones", [128, HL], CDT, kind="ExternalInput").ap()
    out = nc.dram_tensor("out", [T // 2, C], F32, kind="ExternalOutput").ap()

    with tile.TileContext(nc) as tc:
        _emit(nc, tc, xT, wqk, wv, wproj, bias2, ones_r, ident, negtri,
              vones, out)

    nc.compile()
    return nc


def _emit(nc, tc, xT, wqk, wv, wproj, bias2, ones_r, ident, negtri, vones,
          out):
    with (
        tc.tile_pool(name="persist", bufs=1) as pp,
        tc.tile_pool(name="xtp", bufs=16) as xtp,
        tc.tile_pool(name="ps_s", bufs=2, space="PSUM") as ps_s,
        tc.tile_pool(name="ps_av", bufs=4, space="PSUM") as ps_av,
        tc.tile_pool(name="expp", bufs=4) as expp,
        tc.tile_pool(name="atp", bufs=2) as atp,
        tc.tile_pool(name="nrm", bufs=4) as nrm,
        tc.tile_pool(name="stg", bufs=2) as stg,
        tc.tile_pool(name="drp", bufs=12, space="DRAM") as drp,
    ):
        # qkT[jt]: channels 128*jt..128*jt+127 (j<512: Q; j>=512: K), [128, T]
        qkT = [pp.tile([128, T], CDT, name=f"qkT{j}") for j in range(8)]
        # VV[tb]: [128 t, HL heads, D+1] - col D is the ones column (sumexp)
        VV = [pp.tile([128, HL, D + 1], CDT, name=f"VV{t}") for t in range(T // 128)]
        wqk_t = [pp.tile([128, 1024], CDT, name=f"wqk{i}") for i in range(8)]
        wv_t = [pp.tile([128, CL], CDT, name=f"wv{i}") for i in range(8)]
        wproj_t8 = [pp.tile([128, C], CDT, name=f"wproj{i}") for i in range(8)]
        bias_t = pp.tile([1, C], CDT, name="bias_t")
        ones_t = pp.tile([1, 128], CDT, name="ones_t")
        ident_t = pp.tile([128, 128], CDT, name="ident_t")
        negtri_t = pp.tile([128, 128], CDT, name="negtri_t")

        # ---- input loads.  Critical path: xt tch0 + wqk (gates phase A0).
        # Issue them first on the sync queue (round-robins 8 HW rings);
        # everything else from otherwise-idle engine queues so issue cost
        # (~0.65us each, serial per engine) overlaps.
        xt_all = {0: [], 1: []}
        for cb in range(8):
            x_t = xtp.tile([128, 1024], CDT, tag="xt", name=f"xt0_{cb}")
            nc.sync.dma_start(out=x_t, in_=xT[cb, :, 0:1024])
            xt_all[0].append(x_t)
            nc.sync.dma_start(out=wqk_t[cb], in_=wqk[cb])
        for cb in range(8):
            x_t = xtp.tile([128, 1024], CDT, tag="xt", name=f"xt1_{cb}")
            nc.gpsimd.dma_start(out=x_t, in_=xT[cb, :, 1024:2048])
            xt_all[1].append(x_t)
        for i in range(8):
            nc.gpsimd.dma_start(out=wproj_t8[i], in_=wproj[i])
        for i in range(8):
            nc.scalar.dma_start(out=wv_t[i], in_=wv[i])
        nc.scalar.dma_start(out=negtri_t, in_=negtri)
        nc.scalar.dma_start(out=ident_t, in_=ident)
        nc.scalar.dma_start(out=bias_t, in_=bias2)
        nc.scalar.dma_start(out=ones_t, in_=ones_r)
        # ones columns of VV (constant; disjoint from the computed cols)
        for gtb in range(16):
            nc.scalar.dma_start(
                out=VV[gtb][:, :, D:D + 1],
                in_=vones.rearrange("p (h o) -> p h o", o=1))

        at_all, ags = {}, {}
        with tc.tile_critical():
            rid = nc.sync.partition_id()
            rankoff = (rid % 2) * (QC // 2)

        # ---- QKV phase units (emitted piecemeal, interleaved into attention)
        def qk_unit(tch, jt, s5):
            t0 = tch * 1024
            xt = xt_all[tch]
            ps = ps_s.tile([128, N5], F32, tag="s", name=f"pqk{tch}{jt}{s5}")
            for cb in range(8):
                nc.tensor.matmul(
                    ps, wqk_t[cb][:, jt * 128:(jt + 1) * 128],
                    xt[cb][:, s5 * N5:(s5 + 1) * N5],
                    start=(cb == 0), stop=(cb == 7))
            nc.vector.tensor_copy(
                qkT[jt][:, t0 + s5 * N5: t0 + (s5 + 1) * N5], ps)

        def v_unit(gtb):
            tch, tb = gtb // 8, gtb % 8
            xt = xt_all[tch]
            ps = ps_s.tile([128, CL], F32, tag="s", name=f"pv{gtb}")
            for cb in range(8):
                nc.tensor.matmul(
                    ps, xt[cb][:, tb * 128:(tb + 1) * 128], wv_t[cb],
                    start=(cb == 0), stop=(cb == 7))
            nc.vector.tensor_copy(
                VV[gtb][:, :, 0:D],
                ps.rearrange("p (h d) -> p h d", h=HL))

        def emit_at_exchange(pqc):
            pat = at_all[pqc]
            ad = drp.tile([CL, QC], CDT, tag="atdram", name=f"atd{pqc}")
            for ci in range(4):
                nc.sync.dma_start(
                    out=ad[ci * 128:(ci + 1) * 128, :], in_=pat[ci])
            ag = drp.tile([2, CL, QC], CDT, tag="atgdram", name=f"atg{pqc}")
            nc.gpsimd.collective_compute(
                "AllGather", mybir.AluOpType.bypass,
                replica_groups=[[0, 1], [2, 3], [4, 5], [6, 7]],
                ins=[ad[:]], outs=[ag[:]])
            ags[pqc] = ag

        def emit_proj_piece(pqc):
            """proj + output for MY 256 rows (rank offset) of chunk pqc."""
            ag = ags[pqc].rearrange("r c t -> (r c) t")
            agt = [stg.tile([128, QC // 2], CDT, tag=f"agt{ci8}",
                            name=f"agt{pqc}_{ci8}")
                   for ci8 in range(8)]
            for ci8 in range(8):
                nc.sync.dma_start(
                    out=agt[ci8],
                    in_=ag[ci8 * 128:(ci8 + 1) * 128,
                           bass.ds(rankoff, QC // 2)])
            for tt in range(QC // 256):
                st = stg.tile([128, C], F32, tag="stage", name=f"stg{pqc}_{tt}")
                for jc in range(2):
                    pp_ps = ps_s.tile([128, N5], F32, tag="s",
                                      name=f"pp{pqc}_{tt}_{jc}")
                    for ci8 in range(8):
                        nc.tensor.matmul(
                            pp_ps, agt[ci8][:, tt * 128:(tt + 1) * 128],
                            wproj_t8[ci8][:, jc * N5:(jc + 1) * N5],
                            start=(ci8 == 0), stop=False)
                    nc.tensor.matmul(
                        pp_ps, ones_t, bias_t[0:1, jc * N5:(jc + 1) * N5],
                        start=False, stop=True)
                    nc.vector.tensor_copy(st[:, jc * N5:(jc + 1) * N5], pp_ps)
                r0 = pqc * (QC // 2) + tt * 128
                # two half-row DMAs so the 512KB write spreads over 2 rings
                nc.sync.dma_start(out=out[r0:r0 + 128, 0:N5],
                                  in_=st[:, 0:N5])
                nc.sync.dma_start(out=out[r0:r0 + 128, N5:C],
                                  in_=st[:, N5:C])

        def emit_attention(qc, interleave):
            """attention for chunk qc; `interleave` maps hp -> list of fns."""
            q0 = qc * QC
            nkt = (q0 + QC) // KT
            at = [atp.tile([128, QC], CDT, tag=f"at{ci}", name=f"at{qc}_{ci}")
                  for ci in range(4)]
            at_all[qc] = at
            for hp in range(HL // 2):
                heads = (2 * hp, 2 * hp + 1)
                av = {h: ps_av.tile([D + 1, N5], F32, tag="av",
                                    name=f"av{qc}_{h}")
                      for h in heads}
                exps = {}

                def emit_scores(kt):
                    k0 = kt * KT
                    est = max(0, k0 - q0)
                    diag = k0 >= q0
                    # pair-shared score tile: head h at free half h%2
                    sp = ps_s.tile([128, 2, N5], F32, tag="s",
                                   name=f"s{qc}_{hp}_{kt}")
                    for h in heads:
                        roff = (h % 2) * D
                        nc.tensor.matmul(
                            sp[:, h % 2, est:N5],
                            qkT[4 + h // 2][roff:roff + D, k0:k0 + KT],
                            qkT[h // 2][roff:roff + D, q0 + est:q0 + QC],
                            start=True, stop=not diag)
                        if diag:  # mask the upper triangle of the slab
                            nc.tensor.matmul(
                                sp[:, h % 2, est:est + KT],
                                ident_t, negtri_t,
                                start=False, stop=True)
                    ex = expp.tile([128, 2, N5], CDT, tag="exp",
                                   name=f"ex{qc}_{hp}_{kt}")
                    nc.scalar.activation(
                        ex[:, :, est:N5], sp[:, :, est:N5],
                        AF.Exp, scale=0.125)
                    exps[kt] = ex

                def emit_attnv(kt):
                    k0 = kt * KT
                    cst = max(0, k0 - q0)
                    ex = exps.pop(kt)
                    for h in heads:
                        nc.tensor.matmul(
                            av[h][:, cst:N5], VV[kt][:, h, :],
                            ex[:, h % 2, cst:N5],
                            start=(kt == 0), stop=(kt == nkt - 1))

                emit_scores(0)
                for kt in range(1, nkt):
                    emit_scores(kt)
                    emit_attnv(kt - 1)
                emit_attnv(nkt - 1)

                for fn in interleave.get(hp, []):
                    fn()

                for h in heads:
                    roff = (h % 2) * D
                    a = av[h]
                    # custom-DVE/gpsimd ops need partition-0-aligned inputs;
                    # plain DVE copy handles the 64->0 shift (PSUM read)
                    rc0 = nrm.tile([1, N5], F32, tag="rc0",
                                   name=f"rc0{qc}_{h}")
                    nc.vector.tensor_copy(rc0, a[D:D + 1, :])
                    rc = nrm.tile([1, N5], F32, tag="rc", name=f"rc{qc}_{h}")
                    nc.vector.reciprocal_approx_fast(out=rc, in_=rc0)
                    rb = nrm.tile([D, N5], F32, tag="rb", name=f"rb{qc}_{h}")
                    nc.gpsimd.partition_broadcast(rb, rc)
                    nc.vector.tensor_mul(at[h // 2][roff:roff + D, :],
                                         a[0:D, :], rb)
            emit_at_exchange(qc)

        # ---- schedule -----------------------------------------------------
        # bootstrap: Q,K first token-half + V tiles 0-3 (what qc0 needs)
        for jt in range(8):
            qk_unit(0, jt, 0)
        for tb in range(4):
            v_unit(tb)

        def Q(tch, jt, s5):
            return lambda: qk_unit(tch, jt, s5)

        def V(gtb):
            return lambda: v_unit(gtb)

        emit_attention(0, {
            0: [Q(0, 0, 1), Q(0, 4, 1), V(4)],
            1: [Q(0, 1, 1), Q(0, 5, 1), V(5)],
            2: [Q(0, 2, 1), Q(0, 6, 1), V(6)],
            3: [Q(0, 3, 1), Q(0, 7, 1), V(7)],
        })
        emit_attention(1, {
            0: [Q(1, 0, 0), Q(1, 4, 0), V(8)],
            1: [Q(1, 1, 0), Q(1, 5, 0), V(9)],
            2: [Q(1, 2, 0), Q(1, 6, 0), V(10)],
            3: [Q(1, 3, 0), Q(1, 7, 0), V(11),
                lambda: emit_proj_piece(0)],
        })
        emit_attention(2, {
            0: [Q(1, 0, 1), Q(1, 4, 1), V(12)],
            1: [Q(1, 1, 1), Q(1, 5, 1), V(13)],
            2: [Q(1, 2, 1), Q(1, 6, 1), V(14)],
            3: [Q(1, 3, 1), Q(1, 7, 1), V(15),
                lambda: emit_proj_piece(1)],
        })
        emit_attention(3, {
            2: [lambda: emit_proj_piece(2)],
        })
        emit_proj_piece(3)


def _prepare_in_maps(x, Wqkv, Wproj, bproj):
    x = np.asarray(x, dtype=np.float32)
    Wqkv = np.asarray(Wqkv, dtype=np.float32)
    Wproj = np.asarray(Wproj, dtype=np.float32)
    bproj = np.asarray(bproj, dtype=np.float32)

    # negative mask slab: -1e9 where q < k (col < row), 0 where kept
    k_i = np.arange(128)[:, None]
    q_i = np.arange(128)[None, :]
    negtri = np.where(q_i >= k_i, np.float32(0.0), np.float32(-1e9))
    negtri = np.ascontiguousarray(negtri, dtype=np.float32)

    ident = np.eye(128, dtype=np.float32)
    ones_r = np.ones((1, 128), dtype=np.float32)
    vones = np.ones((128, HL), dtype=np.float32)

    in_maps = []
    for core in range(8):
        b, hg = core // 2, core % 2
        xT = np.ascontiguousarray(x[b].T).reshape(8, 128, T)
        wq = Wqkv[:, hg * CL:(hg + 1) * CL]
        wk = Wqkv[:, C + hg * CL: C + (hg + 1) * CL]
        wv_ = Wqkv[:, 2 * C + hg * CL: 2 * C + (hg + 1) * CL]
        wqk = np.ascontiguousarray(
            np.concatenate([wq, wk], axis=1)).reshape(8, 128, 1024)
        wv = np.ascontiguousarray(wv_).reshape(8, 128, CL)
        wp = np.ascontiguousarray(Wproj).reshape(8, 128, C)
        bf = ml_dtypes.bfloat16
        in_maps.append({
            "xT": xT.astype(bf), "wqk": wqk.astype(bf), "wv": wv.astype(bf),
            "wproj": wp.astype(bf),
            "bias2": bproj.reshape(1, C).astype(bf),
            "ones_r": ones_r.astype(bf), "ident": ident.astype(bf),
            "negtri": negtri.astype(bf), "vones": vones.astype(bf),
        })
    return in_maps


def _assemble(results):
    full = np.empty((B, T, C), dtype=np.float32)
    for core in range(8):
        b, r = core // 2, core % 2
        o = results[core]["out"]  # [1024, 1024]
        for qc in range(NQC):
            g0 = qc * QC + r * (QC // 2)
            full[b, g0:g0 + QC // 2] = \
                o[qc * (QC // 2):(qc + 1) * (QC // 2)]
    return full


_NC_CACHE = None


def kernel(x, Wqkv, Wproj, bproj):
    global _NC_CACHE
    if _NC_CACHE is None:
        _NC_CACHE = _build()
    in_maps = _prepare_in_maps(x, Wqkv, Wproj, bproj)
    res = run_bass_kernel_spmd(_NC_CACHE, in_maps, list(range(8)))
    return _assemble(res.results)
